# revision 25
# baseline (speedup 1.0000x reference)
"""Trainium2 Bass kernel for DeepGate3-style attention segment pooling.

Computation (per tensor t in {hs, hf}):
    x = tok_t[member_idx]                  # [E, D] gather
    l = x @ w_t                            # [E]
    attn = softmax(l) within each segment  # segment_ids sorted, G segments
    out_t[g] = sum_{e in seg g} attn_e * x_e   # [G, D]

Strategy (8 cores, full I/O), v7:
  - softmax shift-invariance: attn = exp(l)/segsum(exp(l)).
  - cores get member-balanced contiguous segment ranges; members are
    stream-packed into 128-slot chunks, SPLITTING segments at chunk
    boundaries (partial windows recombined on the host via shipped 1/z).
  - x2 [128, nchunks*256] bf16 (slot-partition member rows, hs|hf) feeds
    the PE segment-sum matmuls for ALL chunks.
  - logits: the kernel is DMA-bound, so only chunks [Q, NBS) of each
    super ship the transposed xt slices for the PE matvec path; chunks
    [0, Q) instead reuse x2 via the DVE fused multiply-accumulate
    (scalar_tensor_tensor accum_out) -- trading idle-DVE time
    (~0.63us/pair) for HBM bytes (~0.17us/pair).
  - exp on ACT, S = exp * mask on GPSIMD, main + z matmuls on PE,
    1/z + out*(1/z) on DVE, bf16 store.
  - TWO-DEEP drain pipeline: super u's PE matmuls are emitted at
    iteration u+1 and its drain (recip/normalize/store) at iteration
    u+2, so no engine stream ever queues behind an in-flight
    cross-engine dependency chain.
  - host recombines [D, cols] windows back to [G, D] using z = 1/zr.
"""

import os

import numpy as np
import ml_dtypes

D = 128          # token dim (hard assumption throughout)
G_DEFAULT = 20000
NCORES_DEFAULT = 8
W_BIN = 8        # max segments per chunk (S window width)
CHUNK = 128      # members per chunk == PE contraction dim
NBS = 32         # chunks per super-group
DUMMY_REL = 15.0

_BF16 = ml_dtypes.bfloat16


def _prep_host(member_idx, segment_ids, G, ncores):
    """Stream packing with segment splitting; member-balanced cores."""
    seg_start = np.searchsorted(segment_ids, np.arange(G + 1)).astype(np.int64)
    counts = np.diff(seg_start)
    E = len(segment_ids)
    cum = np.cumsum(counts)
    bounds = [0]
    for c in range(1, ncores):
        bounds.append(int(np.searchsorted(cum, c * E / ncores)))
    bounds.append(G)

    per_core = []
    nbins_max = 0
    for c in range(ncores):
        chunks = []          # per chunk: [(gseg, off_in_seg, len), ...]
        cur, used = [], 0
        for s in range(bounds[c], bounds[c + 1]):
            n = int(counts[s])
            if n == 0:
                continue
            off = 0
            while n > 0:
                if used == CHUNK or len(cur) == W_BIN:
                    chunks.append(cur)
                    cur, used = [], 0
                take = min(n, CHUNK - used)
                cur.append((s, off, take))
                used += take
                off += take
                n -= take
        if cur:
            chunks.append(cur)
        per_core.append(chunks)
        nbins_max = max(nbins_max, len(chunks))

    nchunks = (nbins_max + NBS - 1) // NBS * NBS

    slot_idx = np.zeros((ncores, nchunks, CHUNK), np.int64)
    relseg = np.full((ncores, CHUNK, nchunks), DUMMY_REL, np.float32)
    out_cols, out_segs = [], []
    for c, chunks in enumerate(per_core):
        cols, segs = [], []
        for k, pieces in enumerate(chunks):
            p = 0
            for w, (s, off, nn) in enumerate(pieces):
                a = seg_start[s] + off
                slot_idx[c, k, p:p + nn] = member_idx[a:a + nn]
                relseg[c, p:p + nn, k] = w
                p += nn
                cols.append(k * W_BIN + w)
                segs.append(s)
        out_cols.append(np.asarray(cols, np.int64))
        out_segs.append(np.asarray(segs, np.int64))
    return nchunks, slot_idx, relseg, out_cols, out_segs


def _build_bass(nchunks, ncores, q):
    """q: chunks per super whose logits run on DVE (no xt shipped)."""
    import concourse.bacc as bacc
    import concourse.tile as tile
    import concourse.mybir as mybir

    f32 = mybir.dt.float32
    bf16 = mybir.dt.bfloat16
    AF = mybir.ActivationFunctionType
    OP = mybir.AluOpType

    nsuper = nchunks // NBS
    ocols = nchunks * W_BIN
    SW = NBS * W_BIN
    NP = NBS - q                           # chunks on the PE/xt logit path

    nc = bacc.Bacc("TRN2", target_bir_lowering=False, debug=False,
                   num_devices=ncores)

    x2_d = nc.dram_tensor("x2", [CHUNK, nchunks * 2 * D], bf16,
                          kind="ExternalInput")
    xt_d = nc.dram_tensor("xt", [D, nsuper * NP * 2 * CHUNK], bf16,
                          kind="ExternalInput") if NP else None
    w2_d = nc.dram_tensor("w2", [CHUNK, 2 * D], bf16, kind="ExternalInput")
    wcol_d = nc.dram_tensor("wcol", [D, 2], bf16, kind="ExternalInput")
    mask_d = nc.dram_tensor("mask", [CHUNK, nchunks * W_BIN], bf16,
                            kind="ExternalInput")
    out_d = {t: nc.dram_tensor(f"out_{t}", [D, ocols], bf16,
                               kind="ExternalOutput") for t in ("hs", "hf")}
    z_d = nc.dram_tensor("z", [1, nsuper * 2 * SW], f32,
                         kind="ExternalOutput")

    with tile.TileContext(nc) as tc:
        with (
            tc.tile_pool(name="const", bufs=1) as constp,
            tc.tile_pool(name="xs", bufs=3) as xsp,
            tc.tile_pool(name="xt", bufs=3) as xtp,
            tc.tile_pool(name="lg", bufs=2) as lgp,
            tc.tile_pool(name="sg", bufs=2) as sgp,
            tc.tile_pool(name="drain", bufs=2) as drainp,
            tc.tile_pool(name="psx", bufs=3, space="PSUM") as psxp,
            tc.tile_pool(name="psz", bufs=3, space="PSUM") as pszp,
            tc.tile_pool(name="psl", bufs=2, space="PSUM") as pslp,
        ):
            w2_sb = constp.tile([CHUNK, 2 * D], bf16, tag="w2")
            nc.sync.dma_start(out=w2_sb[:], in_=w2_d.ap())
            wcol_sb = constp.tile([D, 2], bf16, tag="wcol")
            nc.sync.dma_start(out=wcol_sb[:], in_=wcol_d.ap())
            ones_sb = constp.tile([CHUNK, CHUNK], bf16, tag="ones")
            nc.vector.memset(ones_sb[:], 1.0)

            def emit_main(u, x2, s_ts):
                """Segment-sum + z matmuls for super u (PE only)."""
                psum_x = psxp.tile([CHUNK, 2, SW], f32, tag="px")
                for ti in range(2):
                    for k in range(NBS):
                        nc.tensor.matmul(
                            out=psum_x[:, ti, k * W_BIN:(k + 1) * W_BIN],
                            lhsT=x2[:, k, ti * D:(ti + 1) * D],
                            rhs=s_ts[ti][:, k, :], start=True, stop=True)
                psum_z = pszp.tile([CHUNK, 2, SW], f32, tag="pz")
                for ti in range(2):
                    nc.tensor.matmul(
                        out=psum_z[:, ti, :], lhsT=ones_sb[:],
                        rhs=s_ts[ti][:].rearrange("p a b -> p (a b)"),
                        start=True, stop=True)
                return psum_x, psum_z

            def emit_drain(u, psum_x, psum_z):
                """Normalize + store for super u (emitted 2 supers later).
                Dead (never-read) columns produce inf/NaN and are
                discarded on the host."""
                zr = drainp.tile([CHUNK, 2, SW], f32, tag="zr")
                nc.vector.reciprocal_approx_fast(
                    out=zr[:].rearrange("p a b -> p (a b)"),
                    in_=psum_z[:].rearrange("p a b -> p (a b)"))
                # ship 1/z (row 0); host inverts it back -- the approx
                # error cancels exactly for unsplit segments.
                nc.sync.dma_start(
                    out=z_d.ap()[:, u * 2 * SW:(u + 1) * 2 * SW],
                    in_=zr[0:1, :, :].rearrange("p a b -> p (a b)"))
                # PSUM -> SBUF on ACT (idle), then normalize on GPSIMD so
                # DVE's stream stays [stt..., recip] only.
                xsb = drainp.tile([CHUNK, 2, SW], f32, tag="xsb")
                nc.scalar.copy(out=xsb[:].rearrange("p a b -> p (a b)"),
                               in_=psum_x[:].rearrange("p a b -> p (a b)"))
                osb = drainp.tile([CHUNK, 2, SW], bf16, tag="ob")
                nc.gpsimd.tensor_tensor(
                    out=osb[:], in0=xsb[:], in1=zr[:], op=OP.mult)
                for ti, t in enumerate(("hs", "hf")):
                    nc.sync.dma_start(
                        out=out_d[t].ap()[:, u * SW:(u + 1) * SW],
                        in_=osb[:, ti, :])

            pending = None
            drain_q = None
            for u in range(nsuper):
                x2 = xsp.tile([CHUNK, NBS, 2 * D], bf16, tag="x2")
                nc.sync.dma_start(
                    out=x2[:].rearrange("p a b -> p (a b)"),
                    in_=x2_d.ap()[:, u * NBS * 2 * D:(u + 1) * NBS * 2 * D])
                if NP:
                    xt = xtp.tile([D, NP, 2, CHUNK], bf16, tag="xt")
                    nc.sync.dma_start(
                        out=xt[:].rearrange("p a b c -> p (a b c)"),
                        in_=xt_d.ap()[:, u * NP * 2 * CHUNK:
                                      (u + 1) * NP * 2 * CHUNK])

                mask = sgp.tile([CHUNK, NBS, W_BIN], bf16, tag="mask")
                nc.sync.dma_start(
                    out=mask[:].rearrange("p a b -> p (a b)"),
                    in_=mask_d.ap()[:, u * SW:(u + 1) * SW])
                # DVE logit chunks [0, q)
                l_sb = lgp.tile([CHUNK, 2 * max(q, 1)], f32, tag="lsb")
                scr_dve = lgp.tile([CHUNK, D], bf16, tag="scrd")
                for k in range(q):
                    for ti in range(2):
                        nc.vector.scalar_tensor_tensor(
                            out=scr_dve[:],
                            in0=x2[:, k, ti * D:(ti + 1) * D],
                            scalar=1.0, op0=OP.mult,
                            in1=w2_sb[:, ti * D:(ti + 1) * D], op1=OP.mult,
                            accum_out=l_sb[:, 2 * k + ti:2 * k + ti + 1])
                # PE logit chunks [q, NBS)
                if NP:
                    psl = pslp.tile([CHUNK, NP, 2], f32, tag="psl")
                    for j in range(NP):
                        for ti in range(2):
                            nc.tensor.matmul(
                                out=psl[:, j, ti:ti + 1],
                                lhsT=xt[:, j, ti, :],
                                rhs=wcol_sb[:, ti:ti + 1],
                                start=True, stop=True)

                expc = sgp.tile([CHUNK, NBS, 2], f32, tag="expc")
                if q:
                    nc.scalar.activation(
                        out=expc[:, 0:q, :].rearrange("p a b -> p (a b)"),
                        in_=l_sb[:, 0:2 * q], func=AF.Exp)
                if NP:
                    nc.scalar.activation(
                        out=expc[:, q:NBS, :].rearrange("p a b -> p (a b)"),
                        in_=psl[:].rearrange("p a b -> p (a b)"), func=AF.Exp)

                s_ts = []
                for ti, t in enumerate(("hs", "hf")):
                    s_t = sgp.tile([CHUNK, NBS, W_BIN], bf16, tag=f"s_{t}",
                                   name=f"s_{t}")
                    nc.gpsimd.tensor_tensor(
                        out=s_t[:], in0=mask[:],
                        in1=expc[:, :, ti:ti + 1]
                            .to_broadcast([CHUNK, NBS, W_BIN]),
                        op=OP.mult)
                    s_ts.append(s_t)

                # two-deep software pipeline: PE matmuls for u-1, drain
                # for u-2 -- every dependency is already satisfied when
                # an engine reaches these in its stream.
                if pending is not None:
                    pu, px2, ps = pending
                    psums = emit_main(pu, px2, ps)
                    if drain_q is not None:
                        emit_drain(*drain_q)
                    drain_q = (pu,) + psums
                pending = (u, x2, s_ts)
            pu, px2, ps = pending
            psums = emit_main(pu, px2, ps)
            if drain_q is not None:
                emit_drain(*drain_q)
            emit_drain(pu, *psums)
    nc.compile()
    return nc


def kernel(tf_hs, tf_hf, w_hs, w_hf, member_idx, segment_ids,
           _G=G_DEFAULT, _ncores=NCORES_DEFAULT, _trace=False, _sim=False):
    from concourse.bass_utils import run_bass_kernel_spmd

    tf_hs = np.asarray(tf_hs)
    tf_hf = np.asarray(tf_hf)
    w_hs = np.asarray(w_hs)
    w_hf = np.asarray(w_hf)
    member_idx = np.asarray(member_idx)
    segment_ids = np.asarray(segment_ids)

    assert tf_hs.shape[1] == D
    ncores = _ncores
    G = _G
    q = int(os.environ.get("KERNEL_Q", "12"))

    nchunks, slot_idx, relseg, out_cols, out_segs = _prep_host(
        member_idx, segment_ids, G, ncores)
    nsuper = nchunks // NBS
    SW = NBS * W_BIN
    NP = NBS - q

    nc = _build_bass(nchunks, ncores, q)

    w2f = np.concatenate([w_hs, w_hf]).astype(np.float32)       # [256]
    tok2 = np.concatenate([tf_hs, tf_hf], axis=1).astype(_BF16)  # [N, 256]
    w2_rep = np.ascontiguousarray(np.broadcast_to(w2f.astype(_BF16),
                                                  (CHUNK, 2 * D)))
    wcol = np.ascontiguousarray(
        np.stack([w_hs, w_hf], axis=1).astype(np.float32).astype(_BF16))
    wvals = np.arange(W_BIN, dtype=np.float32)

    kk = np.arange(nchunks) % NBS
    pe_sel = kk >= q                       # chunks that also ship xt slices

    in_maps = []
    for c in range(ncores):
        g = tok2[slot_idx[c]]                     # [nchunks, 128, 256]
        mask = (relseg[c][:, :, None] == wvals).astype(_BF16)
        m = {"x2": np.ascontiguousarray(
                 g.transpose(1, 0, 2).reshape(CHUNK, -1)),
             "w2": w2_rep, "wcol": wcol,
             "mask": np.ascontiguousarray(mask.reshape(CHUNK, -1))}
        if NP:
            gp = g[pe_sel].reshape(nsuper, NP, CHUNK, 2, D)
            # -> [D, nsuper, NP, 2, CHUNK]
            m["xt"] = np.ascontiguousarray(
                gp.transpose(4, 0, 1, 3, 2).reshape(D, -1))
        in_maps.append(m)

    if _sim:
        from concourse.bass_interp import MultiCoreSim
        sim = MultiCoreSim(nc, num_cores=ncores, trace=False,
                           require_finite=False, require_nnan=False)
        for ci in range(ncores):
            core = sim.cores[ci]
            for name, arr in in_maps[ci].items():
                core.tensor(name)[:] = arr
        sim.simulate(check_with_hw=False)
        results = [{nm: np.array(sim.cores[c].tensor(nm))
                    for nm in ("out_hs", "out_hf", "z")}
                   for c in range(ncores)]
    else:
        res = run_bass_kernel_spmd(nc, in_maps, core_ids=list(range(ncores)),
                                   trace=_trace)
        results = res.results
        kernel.last_results = res

    hop = {t: np.zeros((G, D), np.float32) for t in ("hs", "hf")}
    for c in range(ncores):
        zarr = results[c]["z"].reshape(nsuper, 2, SW)
        with np.errstate(divide="ignore", over="ignore"):
            zarr = 1.0 / zarr                         # zr -> z
        cols, segs = out_cols[c], out_segs[c]
        useg, first = np.unique(segs, return_index=True)
        for ti, t in enumerate(("hs", "hf")):
            z = zarr[:, ti, :].reshape(-1)            # [ocols] f32
            o = results[c][f"out_{t}"]                # [D, ocols] bf16
            zc = z[cols]
            U = o[:, cols].astype(np.float32) * zc[None, :]
            Uagg = np.add.reduceat(U.T, first, axis=0)
            zagg = np.add.reduceat(zc, first)
            hop[t][useg] = Uagg / np.maximum(zagg, 1e-9)[:, None]
    return hop["hs"], hop["hf"]


kernel.last_results = None


# revision 26
# speedup vs baseline: 1.0041x; 1.0041x over previous
"""Trainium2 Bass kernel for DeepGate3-style attention segment pooling.

Computation (per tensor t in {hs, hf}):
    x = tok_t[member_idx]                  # [E, D] gather
    l = x @ w_t                            # [E]
    attn = softmax(l) within each segment  # segment_ids sorted, G segments
    out_t[g] = sum_{e in seg g} attn_e * x_e   # [G, D]

Strategy (8 cores, full I/O), v7:
  - softmax shift-invariance: attn = exp(l)/segsum(exp(l)).
  - cores get member-balanced contiguous segment ranges; members are
    stream-packed into 128-slot chunks, SPLITTING segments at chunk
    boundaries (partial windows recombined on the host via shipped 1/z).
  - x2 [128, nchunks*256] bf16 (slot-partition member rows, hs|hf) feeds
    the PE segment-sum matmuls for ALL chunks.
  - logits: the kernel is DMA-bound, so only chunks [Q, NBS) of each
    super ship the transposed xt slices for the PE matvec path; chunks
    [0, Q) instead reuse x2 via the DVE fused multiply-accumulate
    (scalar_tensor_tensor accum_out) -- trading idle-DVE time
    (~0.63us/pair) for HBM bytes (~0.17us/pair).
  - exp on ACT, S = exp * mask on GPSIMD, main + z matmuls on PE,
    1/z + out*(1/z) on DVE, bf16 store.
  - TWO-DEEP drain pipeline: super u's PE matmuls are emitted at
    iteration u+1 and its drain (recip/normalize/store) at iteration
    u+2, so no engine stream ever queues behind an in-flight
    cross-engine dependency chain.
  - host recombines [D, cols] windows back to [G, D] using z = 1/zr.
"""

import os

import numpy as np
import ml_dtypes

D = 128          # token dim (hard assumption throughout)
G_DEFAULT = 20000
NCORES_DEFAULT = 8
W_BIN = 8        # max segments per chunk (S window width)
CHUNK = 128      # members per chunk == PE contraction dim
NBS = 32         # chunks per super-group
DUMMY_REL = 15.0

_BF16 = ml_dtypes.bfloat16


def _prep_host(member_idx, segment_ids, G, ncores):
    """Stream packing with segment splitting; member-balanced cores."""
    seg_start = np.searchsorted(segment_ids, np.arange(G + 1)).astype(np.int64)
    counts = np.diff(seg_start)
    E = len(segment_ids)
    cum = np.cumsum(counts)
    bounds = [0]
    for c in range(1, ncores):
        bounds.append(int(np.searchsorted(cum, c * E / ncores)))
    bounds.append(G)

    per_core = []
    nbins_max = 0
    for c in range(ncores):
        chunks = []          # per chunk: [(gseg, off_in_seg, len), ...]
        cur, used = [], 0
        for s in range(bounds[c], bounds[c + 1]):
            n = int(counts[s])
            if n == 0:
                continue
            off = 0
            while n > 0:
                if used == CHUNK or len(cur) == W_BIN:
                    chunks.append(cur)
                    cur, used = [], 0
                take = min(n, CHUNK - used)
                cur.append((s, off, take))
                used += take
                off += take
                n -= take
        if cur:
            chunks.append(cur)
        per_core.append(chunks)
        nbins_max = max(nbins_max, len(chunks))

    nchunks = (nbins_max + NBS - 1) // NBS * NBS

    slot_idx = np.zeros((ncores, nchunks, CHUNK), np.int64)
    relseg = np.full((ncores, CHUNK, nchunks), DUMMY_REL, np.float32)
    out_cols, out_segs = [], []
    for c, chunks in enumerate(per_core):
        cols, segs = [], []
        for k, pieces in enumerate(chunks):
            p = 0
            for w, (s, off, nn) in enumerate(pieces):
                a = seg_start[s] + off
                slot_idx[c, k, p:p + nn] = member_idx[a:a + nn]
                relseg[c, p:p + nn, k] = w
                p += nn
                cols.append(k * W_BIN + w)
                segs.append(s)
        out_cols.append(np.asarray(cols, np.int64))
        out_segs.append(np.asarray(segs, np.int64))
    return nchunks, slot_idx, relseg, out_cols, out_segs


def _build_bass(nchunks, ncores, q):
    """q: chunks per super whose logits run on DVE (no xt shipped)."""
    import concourse.bacc as bacc
    import concourse.tile as tile
    import concourse.mybir as mybir

    f32 = mybir.dt.float32
    bf16 = mybir.dt.bfloat16
    AF = mybir.ActivationFunctionType
    OP = mybir.AluOpType

    nsuper = nchunks // NBS
    ocols = nchunks * W_BIN
    SW = NBS * W_BIN
    NP = NBS - q                           # chunks on the PE/xt logit path

    nc = bacc.Bacc("TRN2", target_bir_lowering=False, debug=False,
                   num_devices=ncores)

    x2_d = nc.dram_tensor("x2", [CHUNK, nchunks * 2 * D], bf16,
                          kind="ExternalInput")
    xt_d = nc.dram_tensor("xt", [D, nsuper * NP * 2 * CHUNK], bf16,
                          kind="ExternalInput") if NP else None
    w2_d = nc.dram_tensor("w2", [CHUNK, 2 * D], bf16, kind="ExternalInput")
    wcol_d = nc.dram_tensor("wcol", [D, 2], bf16, kind="ExternalInput")
    mask_d = nc.dram_tensor("mask", [CHUNK, nchunks * W_BIN], bf16,
                            kind="ExternalInput")
    out_d = {t: nc.dram_tensor(f"out_{t}", [D, ocols], bf16,
                               kind="ExternalOutput") for t in ("hs", "hf")}
    z_d = nc.dram_tensor("z", [1, nsuper * 2 * SW], f32,
                         kind="ExternalOutput")

    with tile.TileContext(nc) as tc:
        with (
            tc.tile_pool(name="const", bufs=1) as constp,
            tc.tile_pool(name="xs", bufs=3) as xsp,
            tc.tile_pool(name="xt", bufs=3) as xtp,
            tc.tile_pool(name="lg", bufs=2) as lgp,
            tc.tile_pool(name="sg", bufs=2) as sgp,
            tc.tile_pool(name="drain", bufs=2) as drainp,
            tc.tile_pool(name="psx", bufs=3, space="PSUM") as psxp,
            tc.tile_pool(name="psz", bufs=3, space="PSUM") as pszp,
            tc.tile_pool(name="psl", bufs=2, space="PSUM") as pslp,
        ):
            w2_sb = constp.tile([CHUNK, 2 * D], bf16, tag="w2")
            nc.sync.dma_start(out=w2_sb[:], in_=w2_d.ap())
            wcol_sb = constp.tile([D, 2], bf16, tag="wcol")
            nc.sync.dma_start(out=wcol_sb[:], in_=wcol_d.ap())
            ones_sb = constp.tile([CHUNK, CHUNK], bf16, tag="ones")
            nc.vector.memset(ones_sb[:], 1.0)

            def emit_main(u, x2, s_ts):
                """Segment-sum + z matmuls for super u (PE only)."""
                psum_x = psxp.tile([CHUNK, 2, SW], f32, tag="px")
                for ti in range(2):
                    for k in range(NBS):
                        nc.tensor.matmul(
                            out=psum_x[:, ti, k * W_BIN:(k + 1) * W_BIN],
                            lhsT=x2[:, k, ti * D:(ti + 1) * D],
                            rhs=s_ts[ti][:, k, :], start=True, stop=True)
                psum_z = pszp.tile([CHUNK, 2, SW], f32, tag="pz")
                for ti in range(2):
                    nc.tensor.matmul(
                        out=psum_z[:, ti, :], lhsT=ones_sb[:],
                        rhs=s_ts[ti][:].rearrange("p a b -> p (a b)"),
                        start=True, stop=True)
                return psum_x, psum_z

            def emit_drain(u, psum_x, psum_z):
                """Normalize + store for super u (emitted 2 supers later).
                Dead (never-read) columns produce inf/NaN and are
                discarded on the host."""
                zr = drainp.tile([CHUNK, 2, SW], f32, tag="zr")
                nc.vector.reciprocal_approx_fast(
                    out=zr[:].rearrange("p a b -> p (a b)"),
                    in_=psum_z[:].rearrange("p a b -> p (a b)"))
                # ship 1/z (row 0); host inverts it back -- the approx
                # error cancels exactly for unsplit segments.
                nc.sync.dma_start(
                    out=z_d.ap()[:, u * 2 * SW:(u + 1) * 2 * SW],
                    in_=zr[0:1, :, :].rearrange("p a b -> p (a b)"))
                # PSUM -> SBUF on ACT (idle), then normalize on GPSIMD so
                # DVE's stream stays [stt..., recip] only.
                xsb = drainp.tile([CHUNK, 2, SW], f32, tag="xsb")
                nc.scalar.copy(out=xsb[:].rearrange("p a b -> p (a b)"),
                               in_=psum_x[:].rearrange("p a b -> p (a b)"))
                osb = drainp.tile([CHUNK, 2, SW], bf16, tag="ob")
                nc.gpsimd.tensor_tensor(
                    out=osb[:], in0=xsb[:], in1=zr[:], op=OP.mult)
                for ti, t in enumerate(("hs", "hf")):
                    nc.sync.dma_start(
                        out=out_d[t].ap()[:, u * SW:(u + 1) * SW],
                        in_=osb[:, ti, :])

            pending = None
            drain_q = None
            for u in range(nsuper):
                x2 = xsp.tile([CHUNK, NBS, 2 * D], bf16, tag="x2")
                nc.sync.dma_start(
                    out=x2[:].rearrange("p a b -> p (a b)"),
                    in_=x2_d.ap()[:, u * NBS * 2 * D:(u + 1) * NBS * 2 * D])
                if NP:
                    xt = xtp.tile([D, NP, 2, CHUNK], bf16, tag="xt")
                    nc.sync.dma_start(
                        out=xt[:].rearrange("p a b c -> p (a b c)"),
                        in_=xt_d.ap()[:, u * NP * 2 * CHUNK:
                                      (u + 1) * NP * 2 * CHUNK])

                mask = sgp.tile([CHUNK, NBS, W_BIN], bf16, tag="mask")
                nc.sync.dma_start(
                    out=mask[:].rearrange("p a b -> p (a b)"),
                    in_=mask_d.ap()[:, u * SW:(u + 1) * SW])
                # DVE logit chunks [0, q)
                l_sb = lgp.tile([CHUNK, 2 * max(q, 1)], f32, tag="lsb")
                scr_dve = lgp.tile([CHUNK, D], bf16, tag="scrd")
                for k in range(q):
                    for ti in range(2):
                        nc.vector.scalar_tensor_tensor(
                            out=scr_dve[:],
                            in0=x2[:, k, ti * D:(ti + 1) * D],
                            scalar=1.0, op0=OP.mult,
                            in1=w2_sb[:, ti * D:(ti + 1) * D], op1=OP.mult,
                            accum_out=l_sb[:, 2 * k + ti:2 * k + ti + 1])
                # PE logit chunks [q, NBS)
                if NP:
                    psl = pslp.tile([CHUNK, NP, 2], f32, tag="psl")
                    for j in range(NP):
                        for ti in range(2):
                            nc.tensor.matmul(
                                out=psl[:, j, ti:ti + 1],
                                lhsT=xt[:, j, ti, :],
                                rhs=wcol_sb[:, ti:ti + 1],
                                start=True, stop=True)

                expc = sgp.tile([CHUNK, NBS, 2], f32, tag="expc")
                if q:
                    nc.scalar.activation(
                        out=expc[:, 0:q, :].rearrange("p a b -> p (a b)"),
                        in_=l_sb[:, 0:2 * q], func=AF.Exp)
                if NP:
                    nc.scalar.activation(
                        out=expc[:, q:NBS, :].rearrange("p a b -> p (a b)"),
                        in_=psl[:].rearrange("p a b -> p (a b)"), func=AF.Exp)

                s_ts = []
                for ti, t in enumerate(("hs", "hf")):
                    s_t = sgp.tile([CHUNK, NBS, W_BIN], bf16, tag=f"s_{t}",
                                   name=f"s_{t}")
                    nc.gpsimd.tensor_tensor(
                        out=s_t[:], in0=mask[:],
                        in1=expc[:, :, ti:ti + 1]
                            .to_broadcast([CHUNK, NBS, W_BIN]),
                        op=OP.mult)
                    s_ts.append(s_t)

                # two-deep software pipeline: PE matmuls for u-1, drain
                # for u-2.  The drain is emitted BEFORE emit_main so the
                # scheduler's conservative engine-order semaphores tie it
                # only to this super's early psl matmuls -- NOT to
                # main/z(u-1), which would serialize the whole pipeline.
                if drain_q is not None:
                    emit_drain(*drain_q)
                if pending is not None:
                    pu, px2, ps = pending
                    psums = emit_main(pu, px2, ps)
                    drain_q = (pu,) + psums
                pending = (u, x2, s_ts)
            if drain_q is not None:
                emit_drain(*drain_q)
            pu, px2, ps = pending
            psums = emit_main(pu, px2, ps)
            emit_drain(pu, *psums)
    nc.compile()
    return nc


def kernel(tf_hs, tf_hf, w_hs, w_hf, member_idx, segment_ids,
           _G=G_DEFAULT, _ncores=NCORES_DEFAULT, _trace=False, _sim=False):
    from concourse.bass_utils import run_bass_kernel_spmd

    tf_hs = np.asarray(tf_hs)
    tf_hf = np.asarray(tf_hf)
    w_hs = np.asarray(w_hs)
    w_hf = np.asarray(w_hf)
    member_idx = np.asarray(member_idx)
    segment_ids = np.asarray(segment_ids)

    assert tf_hs.shape[1] == D
    ncores = _ncores
    G = _G
    q = int(os.environ.get("KERNEL_Q", "12"))

    nchunks, slot_idx, relseg, out_cols, out_segs = _prep_host(
        member_idx, segment_ids, G, ncores)
    nsuper = nchunks // NBS
    SW = NBS * W_BIN
    NP = NBS - q

    nc = _build_bass(nchunks, ncores, q)

    w2f = np.concatenate([w_hs, w_hf]).astype(np.float32)       # [256]
    tok2 = np.concatenate([tf_hs, tf_hf], axis=1).astype(_BF16)  # [N, 256]
    w2_rep = np.ascontiguousarray(np.broadcast_to(w2f.astype(_BF16),
                                                  (CHUNK, 2 * D)))
    wcol = np.ascontiguousarray(
        np.stack([w_hs, w_hf], axis=1).astype(np.float32).astype(_BF16))
    wvals = np.arange(W_BIN, dtype=np.float32)

    kk = np.arange(nchunks) % NBS
    pe_sel = kk >= q                       # chunks that also ship xt slices

    in_maps = []
    for c in range(ncores):
        g = tok2[slot_idx[c]]                     # [nchunks, 128, 256]
        mask = (relseg[c][:, :, None] == wvals).astype(_BF16)
        m = {"x2": np.ascontiguousarray(
                 g.transpose(1, 0, 2).reshape(CHUNK, -1)),
             "w2": w2_rep, "wcol": wcol,
             "mask": np.ascontiguousarray(mask.reshape(CHUNK, -1))}
        if NP:
            gp = g[pe_sel].reshape(nsuper, NP, CHUNK, 2, D)
            # -> [D, nsuper, NP, 2, CHUNK]
            m["xt"] = np.ascontiguousarray(
                gp.transpose(4, 0, 1, 3, 2).reshape(D, -1))
        in_maps.append(m)

    if _sim:
        from concourse.bass_interp import MultiCoreSim
        sim = MultiCoreSim(nc, num_cores=ncores, trace=False,
                           require_finite=False, require_nnan=False)
        for ci in range(ncores):
            core = sim.cores[ci]
            for name, arr in in_maps[ci].items():
                core.tensor(name)[:] = arr
        sim.simulate(check_with_hw=False)
        results = [{nm: np.array(sim.cores[c].tensor(nm))
                    for nm in ("out_hs", "out_hf", "z")}
                   for c in range(ncores)]
    else:
        res = run_bass_kernel_spmd(nc, in_maps, core_ids=list(range(ncores)),
                                   trace=_trace)
        results = res.results
        kernel.last_results = res

    hop = {t: np.zeros((G, D), np.float32) for t in ("hs", "hf")}
    for c in range(ncores):
        zarr = results[c]["z"].reshape(nsuper, 2, SW)
        with np.errstate(divide="ignore", over="ignore"):
            zarr = 1.0 / zarr                         # zr -> z
        cols, segs = out_cols[c], out_segs[c]
        useg, first = np.unique(segs, return_index=True)
        for ti, t in enumerate(("hs", "hf")):
            z = zarr[:, ti, :].reshape(-1)            # [ocols] f32
            o = results[c][f"out_{t}"]                # [D, ocols] bf16
            zc = z[cols]
            U = o[:, cols].astype(np.float32) * zc[None, :]
            Uagg = np.add.reduceat(U.T, first, axis=0)
            zagg = np.add.reduceat(zc, first)
            hop[t][useg] = Uagg / np.maximum(zagg, 1e-9)[:, None]
    return hop["hs"], hop["hf"]


kernel.last_results = None
